# revision 1
# baseline (speedup 1.0000x reference)
"""Trainium2 Bass kernel for the ConditionalDDPM forward-diffusion problem.

Computes  xt = sqrt(alpha_bar[t]) * images + sqrt(1 - alpha_bar[t]) * e
for B=65536 images of shape (1, 28, 28), t in [0, 1000).

Strategy (pure data parallel, 8 NeuronCores):
  - Shard images/e/t along batch: 8192 samples per core.
  - Instead of a table gather, the per-sample scalars are computed on device:
    g(t) = ln(alpha_bar[t]) is a smooth near-quartic function of t, fitted by
    a degree-6 zero-intercept polynomial in u=(t+1)/1000 (f64 fit residual
    ~5e-13; full f32 device pipeline matches the f32 reference to ~3e-7
    global rel err).  Per core this is one contiguous 32KB t-load plus ~10
    tiny [128, 64] DVE/ACT ops - ready in a few us, entirely off the
    critical DMA path.  (A dma_gather variant measured ~8.7us of Pool-engine
    ucode per 1024 indices and delayed first compute to ~68us.)
  - Sample layout: sample s = 64*p + i lives at (partition p, unit i);
    unit i's per-partition scalars are a[:, i], b[:, i].
  - Main stream: 16 groups of [128 partitions x 4 units x 784 pixels]
    (1.57MB DMAs, 3136B contiguous per descriptor).  Per unit:
        ACT:  u  = a * x          (activation Copy with per-partition scale)
        DVE:  xt = (b * e) + u    (scalar_tensor_tensor, per-partition scalar)
    Both hide under the ~77 MB/core HBM stream (~215us roofline).
"""

import sys

if "/opt/trn_rl_repo" not in sys.path:
    sys.path.insert(0, "/opt/trn_rl_repo")

import numpy as np

B = 65536
T = 1000
BETA_1 = 1e-4
BETA_T = 0.02
N_CORES = 8
NS = B // N_CORES  # samples per core = 8192
PIX = 784
K = 4  # 128-partition units per DMA group
POLY_DEG = 6

_cache = {}


def g_poly_coeffs() -> np.ndarray:
    """c[0..5] with g(u) ~= (((((c6*u + c5)*u + c4)*u + c3)*u + c2)*u + c1)*u,
    u = (t+1)/1000, g = ln(alpha_bar[t]).  Fit in f64; residual ~5e-13."""
    slope = (BETA_T - BETA_1) / (T - 1)
    betas = BETA_1 + slope * np.arange(T, dtype=np.float64)
    g_exact = np.cumsum(np.log1p(-betas))
    u = (np.arange(T, dtype=np.float64) + 1.0) / 1000.0
    A = np.stack([u**k for k in range(1, POLY_DEG + 1)], axis=1)
    c, *_ = np.linalg.lstsq(A, g_exact, rcond=None)
    return c


def alpha_tables() -> np.ndarray:
    """Reference-exact [T, 2] f32 table (used by test harnesses only)."""
    slope = np.float32((BETA_T - BETA_1) / (T - 1))
    betas = np.float32(BETA_1) + slope * np.arange(T, dtype=np.float32)
    ab = np.cumprod((np.float32(1.0) - betas).astype(np.float32)).astype(np.float32)
    tab = np.zeros((T, 2), dtype=np.float32)
    tab[:, 0] = np.sqrt(ab).astype(np.float32)
    tab[:, 1] = np.sqrt((np.float32(1.0) - ab).astype(np.float32)).astype(np.float32)
    return tab


def build_program(ns: int = NS, k: int = K):
    """Build the per-core Bass program (same NEFF on all 8 cores)."""
    from concourse import bacc, mybir
    import concourse.tile as tile

    assert ns % (128 * k) == 0
    n_units = ns // 128
    n_io = ns // (128 * k)
    f32 = mybir.dt.float32
    Alu = mybir.AluOpType
    Act = mybir.ActivationFunctionType
    coeffs = [float(c) for c in g_poly_coeffs()]

    nc = bacc.Bacc(
        "TRN2",
        target_bir_lowering=False,
        debug=False,
        enable_asserts=False,
        num_devices=N_CORES,
    )
    x = nc.dram_tensor("x", [ns, PIX], f32, kind="ExternalInput").ap()
    y = nc.dram_tensor("y", [ns, PIX], f32, kind="ExternalInput").ap()
    tt = nc.dram_tensor("t", [ns], mybir.dt.int32, kind="ExternalInput").ap()
    out = nc.dram_tensor("out", [ns, PIX], f32, kind="ExternalOutput").ap()

    # sample s = 64*p + 4*io + kk  lives at (group io, partition p, slot kk)
    x_v = x.rearrange("(p io k) m -> io p k m", p=128, io=n_io, k=k)
    y_v = y.rearrange("(p io k) m -> io p k m", p=128, io=n_io, k=k)
    o_v = out.rearrange("(p io k) m -> io p k m", p=128, io=n_io, k=k)
    t_v = tt.rearrange("(p i) -> p i", p=128)  # contiguous 256B per partition

    with tile.TileContext(nc) as tc:
        with (
            tc.tile_pool(name="xs", bufs=6) as xpool,
            tc.tile_pool(name="ys", bufs=6) as ypool,
            tc.tile_pool(name="singles", bufs=1) as singles,
        ):
            # ---- per-sample scalars: a = exp(g/2), b = sqrt(1 - exp(g)) ----
            ti = singles.tile([128, n_units], mybir.dt.int32)
            nc.gpsimd.dma_start(out=ti[:], in_=t_v)
            # u = (t + 1) / 1000   (int32 in, f32 out)
            uu = singles.tile([128, n_units], f32)
            nc.vector.tensor_scalar(
                out=uu[:], in0=ti[:], scalar1=1.0, scalar2=0.001,
                op0=Alu.add, op1=Alu.mult,
            )
            # Horner with zero intercept: h = u*c6; h = (h + c_k)*u, k=5..1
            hh = singles.tile([128, n_units], f32)
            nc.vector.tensor_scalar_mul(out=hh[:], in0=uu[:], scalar1=coeffs[5])
            for kk_ in range(POLY_DEG - 2, -1, -1):
                nc.vector.scalar_tensor_tensor(
                    out=hh[:], in0=hh[:], scalar=coeffs[kk_], in1=uu[:],
                    op0=Alu.add, op1=Alu.mult,
                )
            # a = exp(0.5*g)
            a_t = singles.tile([128, n_units], f32)
            nc.scalar.activation(out=a_t[:], in_=hh[:], func=Act.Exp, scale=0.5)
            # b = sqrt(1 - exp(g))
            b_t = singles.tile([128, n_units], f32)
            nc.scalar.activation(out=b_t[:], in_=hh[:], func=Act.Exp)
            nc.vector.tensor_scalar(
                out=b_t[:], in0=b_t[:], scalar1=1.0, scalar2=-1.0,
                op0=Alu.subtract, op1=Alu.mult,
            )
            nc.scalar.activation(out=b_t[:], in_=b_t[:], func=Act.Sqrt)

            # ---- main stream ----
            for io in range(n_io):
                xt = xpool.tile([128, k, PIX], f32)
                yt = ypool.tile([128, k, PIX], f32)
                if io == n_io - 1:
                    # finer-grained tail: per-unit loads so the last units'
                    # compute can start before the whole group has landed
                    for kk in range(k):
                        nc.sync.dma_start(out=xt[:, kk, :], in_=x_v[io, :, kk, :])
                        nc.sync.dma_start(out=yt[:, kk, :], in_=y_v[io, :, kk, :])
                else:
                    nc.sync.dma_start(out=xt[:], in_=x_v[io, :, :, :])
                    nc.sync.dma_start(out=yt[:], in_=y_v[io, :, :, :])
                for kk in range(k):
                    i = io * k + kk
                    nc.scalar.activation(
                        out=xt[:, kk, :],
                        in_=xt[:, kk, :],
                        func=Act.Copy,
                        scale=a_t[:, i : i + 1],
                    )
                    nc.vector.scalar_tensor_tensor(
                        out=xt[:, kk, :],
                        in0=yt[:, kk, :],
                        scalar=b_t[:, i : i + 1],
                        in1=xt[:, kk, :],
                        op0=Alu.mult,
                        op1=Alu.add,
                    )
                if io == n_io - 1:
                    # finer-grained tail: store each unit as soon as its
                    # compute finishes instead of waiting for the whole group
                    for kk in range(k):
                        nc.scalar.dma_start(out=o_v[io, :, kk, :], in_=xt[:, kk, :])
                else:
                    nc.scalar.dma_start(out=o_v[io, :, :, :], in_=xt[:])

    nc.compile()
    return nc


def make_in_maps(images, e, t):
    x = np.ascontiguousarray(np.asarray(images, dtype=np.float32).reshape(B, PIX))
    yy = np.ascontiguousarray(np.asarray(e, dtype=np.float32).reshape(B, PIX))
    tt = np.ascontiguousarray(np.asarray(t, dtype=np.int32).reshape(B))
    in_maps = []
    for c in range(N_CORES):
        sl = slice(c * NS, (c + 1) * NS)
        in_maps.append(
            {
                "x": np.ascontiguousarray(x[sl]),
                "y": np.ascontiguousarray(yy[sl]),
                "t": np.ascontiguousarray(tt[sl]),
            }
        )
    return in_maps


def _get_runner():
    """Build (once) a jitted shard_map callable over the 8 cores.

    Mirrors concourse.bass2jax.run_bass_via_pjrt, but caches the compiled
    executable so repeated kernel() calls skip retracing/recompiling, and
    keeps the output placeholder buffers resident on device.
    """
    if "runner" in _cache:
        return _cache["runner"]

    import jax
    from jax.sharding import Mesh, PartitionSpec, NamedSharding
    from jax.experimental.shard_map import shard_map
    from concourse import mybir
    from concourse.bass2jax import (
        _bass_exec_p,
        install_neuronx_cc_hook,
        partition_id_tensor,
    )

    nc = _cache.get("nc")
    if nc is None:
        nc = _cache["nc"] = build_program()

    install_neuronx_cc_hook()

    partition_name = nc.partition_id_tensor.name if nc.partition_id_tensor else None
    in_names, out_names, out_avals = [], [], []
    for alloc in nc.m.functions[0].allocations:
        if not isinstance(alloc, mybir.MemoryLocationSet):
            continue
        name = alloc.memorylocations[0].name
        if alloc.kind == "ExternalInput":
            if name != partition_name:
                in_names.append(name)
        elif alloc.kind == "ExternalOutput":
            out_names.append(name)
            out_avals.append(
                jax.core.ShapedArray(tuple(alloc.tensor_shape), mybir.dt.np(alloc.dtype))
            )
    n_params = len(in_names)
    all_names = list(in_names) + out_names
    if partition_name is not None:
        all_names.append(partition_name)

    def _body(*args):
        # args = params + output placeholder buffers (the hook's parameter-
        # order check requires every bass_exec operand to be a jit parameter)
        operands = list(args)
        if partition_name is not None:
            operands.append(partition_id_tensor())
        outs = _bass_exec_p.bind(
            *operands,
            out_avals=tuple(out_avals),
            in_names=tuple(all_names),
            out_names=tuple(out_names),
            lowering_input_output_aliases=(),
            sim_require_finite=True,
            sim_require_nnan=True,
            nc=nc,
        )
        return tuple(outs)

    devices = jax.devices()[:N_CORES]
    assert len(devices) == N_CORES
    mesh = Mesh(np.asarray(devices), ("core",))
    n_outs = len(out_names)
    sharded = jax.jit(
        shard_map(
            _body,
            mesh=mesh,
            in_specs=(PartitionSpec("core"),) * (n_params + n_outs),
            out_specs=(PartitionSpec("core"),) * n_outs,
            check_rep=False,
        ),
        keep_unused=True,
    )
    # Output placeholder buffers: uploaded to device once, NOT donated, so
    # they stay valid and cost nothing on subsequent calls.
    zeros_dev = [
        jax.device_put(
            np.zeros((N_CORES * a.shape[0], *a.shape[1:]), a.dtype),
            NamedSharding(mesh, PartitionSpec("core")),
        )
        for a in out_avals
    ]
    _cache["runner"] = (sharded, in_names, out_names, zeros_dev)
    return _cache["runner"]


def kernel(images, e, t):
    images = np.asarray(images)
    orig_shape = images.shape

    x = np.ascontiguousarray(images.astype(np.float32, copy=False).reshape(B, PIX))
    yy = np.ascontiguousarray(np.asarray(e, dtype=np.float32).reshape(B, PIX))
    tt = np.ascontiguousarray(np.asarray(t, dtype=np.int32).reshape(B))

    try:
        sharded, in_names, out_names, zeros_dev = _get_runner()
        global_in = {"x": x, "y": yy, "t": tt}
        out_arrs = sharded(*[global_in[n] for n in in_names], *zeros_dev)
        full = np.asarray(out_arrs[out_names.index("out")])
    except Exception:
        # Fallback: the stock (slower, but battle-tested) execution path.
        from concourse import bass_utils

        if "nc" not in _cache:
            _cache["nc"] = build_program()
        res = bass_utils.run_bass_kernel_spmd(
            _cache["nc"], make_in_maps(images, e, t), core_ids=list(range(N_CORES))
        )
        full = np.concatenate([res.results[c]["out"] for c in range(N_CORES)], axis=0)

    return full.reshape(orig_shape).astype(np.float32)



# revision 2
# speedup vs baseline: 2.1664x; 2.1664x over previous
"""Trainium2 Bass kernel for the ConditionalDDPM forward-diffusion problem.

Computes  xt = sqrt(alpha_bar[t]) * images + sqrt(1 - alpha_bar[t]) * e
for B=65536 images of shape (1, 28, 28), t in [0, 1000).

Strategy (pure data parallel, 8 NeuronCores):
  - Shard images/e/t along batch: 8192 samples per core.
  - The problem is pure HBM-bandwidth-bound (~358 GB/s/core).  The rel-err
    budget (2e-2) is ~30x looser than fp16 end-to-end error (~6e-4), so the
    bulk tensors are cast to fp16 on the HOST and streamed at half the
    bytes: 2x12.8MB loads + 12.8MB store = 38.5MB/core vs 77MB in f32.
    Roofline ~108us vs ~215us.
  - Per-sample scalars are computed on device from t (no table gather):
    g(t) = ln(alpha_bar[t]) fitted by a degree-6 zero-intercept polynomial
    in u=(t+1)/1000 (f64 fit residual ~5e-13); a = exp(g/2), b = sqrt(1-e^g)
    in f32.  One contiguous 32KB t-load plus ~10 tiny [128, 64] ops.
  - Sample layout: sample s = 64*p + i lives at (partition p, unit i);
    unit i's per-partition scalars are a[:, i], b[:, i].
  - Main stream: 8 groups of [128 partitions x 8 units x 784 pixels] fp16
    (1.6MB DMAs, 12544B contiguous per partition).  Per unit:
        ACT:  u  = a * x          (activation Copy with per-partition scale)
        DVE:  xt = (b * e) + u    (scalar_tensor_tensor, per-partition scalar)
    Both hide under the ~38.5 MB/core HBM stream.
"""

import sys

if "/opt/trn_rl_repo" not in sys.path:
    sys.path.insert(0, "/opt/trn_rl_repo")

import numpy as np

B = 65536
T = 1000
BETA_1 = 1e-4
BETA_T = 0.02
N_CORES = 8
NS = B // N_CORES  # samples per core = 8192
PIX = 784
K = 8  # 128-partition units per DMA group
POLY_DEG = 6

_cache = {}


def g_poly_coeffs() -> np.ndarray:
    """c[0..5] with g(u) ~= (((((c6*u + c5)*u + c4)*u + c3)*u + c2)*u + c1)*u,
    u = (t+1)/1000, g = ln(alpha_bar[t]).  Fit in f64; residual ~5e-13."""
    slope = (BETA_T - BETA_1) / (T - 1)
    betas = BETA_1 + slope * np.arange(T, dtype=np.float64)
    g_exact = np.cumsum(np.log1p(-betas))
    u = (np.arange(T, dtype=np.float64) + 1.0) / 1000.0
    A = np.stack([u**k for k in range(1, POLY_DEG + 1)], axis=1)
    c, *_ = np.linalg.lstsq(A, g_exact, rcond=None)
    return c


def alpha_tables() -> np.ndarray:
    """Reference-exact [T, 2] f32 table (used by test harnesses only)."""
    slope = np.float32((BETA_T - BETA_1) / (T - 1))
    betas = np.float32(BETA_1) + slope * np.arange(T, dtype=np.float32)
    ab = np.cumprod((np.float32(1.0) - betas).astype(np.float32)).astype(np.float32)
    tab = np.zeros((T, 2), dtype=np.float32)
    tab[:, 0] = np.sqrt(ab).astype(np.float32)
    tab[:, 1] = np.sqrt((np.float32(1.0) - ab).astype(np.float32)).astype(np.float32)
    return tab


def build_program(ns: int = NS, k: int = K):
    """Build the per-core Bass program (same NEFF on all 8 cores)."""
    from concourse import bacc, mybir
    import concourse.tile as tile

    assert ns % (128 * k) == 0
    n_units = ns // 128
    n_io = ns // (128 * k)
    f32 = mybir.dt.float32
    f16 = mybir.dt.float16
    Alu = mybir.AluOpType
    Act = mybir.ActivationFunctionType
    coeffs = [float(c) for c in g_poly_coeffs()]

    nc = bacc.Bacc(
        "TRN2",
        target_bir_lowering=False,
        debug=False,
        enable_asserts=False,
        num_devices=N_CORES,
    )
    x = nc.dram_tensor("x", [ns, PIX], f16, kind="ExternalInput").ap()
    y = nc.dram_tensor("y", [ns, PIX], f16, kind="ExternalInput").ap()
    tt = nc.dram_tensor("t", [ns], mybir.dt.int32, kind="ExternalInput").ap()
    out = nc.dram_tensor("out", [ns, PIX], f16, kind="ExternalOutput").ap()

    # sample s = 64*p + 8*io + kk  lives at (group io, partition p, slot kk)
    x_v = x.rearrange("(p io k) m -> io p k m", p=128, io=n_io, k=k)
    y_v = y.rearrange("(p io k) m -> io p k m", p=128, io=n_io, k=k)
    o_v = out.rearrange("(p io k) m -> io p k m", p=128, io=n_io, k=k)
    t_v = tt.rearrange("(p i) -> p i", p=128)  # contiguous 256B per partition

    with tile.TileContext(nc) as tc:
        with (
            tc.tile_pool(name="xs", bufs=4) as xpool,
            tc.tile_pool(name="ys", bufs=4) as ypool,
            tc.tile_pool(name="singles", bufs=1) as singles,
        ):
            # ---- per-sample scalars: a = exp(g/2), b = sqrt(1 - exp(g)) ----
            ti = singles.tile([128, n_units], mybir.dt.int32)
            nc.gpsimd.dma_start(out=ti[:], in_=t_v)
            # u = (t + 1) / 1000   (int32 in, f32 out)
            uu = singles.tile([128, n_units], f32)
            nc.vector.tensor_scalar(
                out=uu[:], in0=ti[:], scalar1=1.0, scalar2=0.001,
                op0=Alu.add, op1=Alu.mult,
            )
            # Horner with zero intercept: h = u*c6; h = (h + c_k)*u, k=5..1
            hh = singles.tile([128, n_units], f32)
            nc.vector.tensor_scalar_mul(out=hh[:], in0=uu[:], scalar1=coeffs[5])
            for kk_ in range(POLY_DEG - 2, -1, -1):
                nc.vector.scalar_tensor_tensor(
                    out=hh[:], in0=hh[:], scalar=coeffs[kk_], in1=uu[:],
                    op0=Alu.add, op1=Alu.mult,
                )
            # a = exp(0.5*g)
            a_t = singles.tile([128, n_units], f32)
            nc.scalar.activation(out=a_t[:], in_=hh[:], func=Act.Exp, scale=0.5)
            # b = sqrt(1 - exp(g))
            b_t = singles.tile([128, n_units], f32)
            nc.scalar.activation(out=b_t[:], in_=hh[:], func=Act.Exp)
            nc.vector.tensor_scalar(
                out=b_t[:], in0=b_t[:], scalar1=1.0, scalar2=-1.0,
                op0=Alu.subtract, op1=Alu.mult,
            )
            nc.scalar.activation(out=b_t[:], in_=b_t[:], func=Act.Sqrt)

            # ---- main stream ----
            for io in range(n_io):
                xt = xpool.tile([128, k, PIX], f16)
                yt = ypool.tile([128, k, PIX], f16)
                if io == n_io - 1:
                    # finer-grained tail: per-unit loads so the last units'
                    # compute can start before the whole group has landed
                    for kk in range(k):
                        nc.sync.dma_start(out=xt[:, kk, :], in_=x_v[io, :, kk, :])
                        nc.sync.dma_start(out=yt[:, kk, :], in_=y_v[io, :, kk, :])
                else:
                    nc.sync.dma_start(out=xt[:], in_=x_v[io, :, :, :])
                    nc.sync.dma_start(out=yt[:], in_=y_v[io, :, :, :])
                for kk in range(k):
                    i = io * k + kk
                    nc.scalar.activation(
                        out=xt[:, kk, :],
                        in_=xt[:, kk, :],
                        func=Act.Copy,
                        scale=a_t[:, i : i + 1],
                    )
                    nc.vector.scalar_tensor_tensor(
                        out=xt[:, kk, :],
                        in0=yt[:, kk, :],
                        scalar=b_t[:, i : i + 1],
                        in1=xt[:, kk, :],
                        op0=Alu.mult,
                        op1=Alu.add,
                    )
                if io == n_io - 1:
                    # finer-grained tail: store each unit as soon as its
                    # compute finishes instead of waiting for the whole group
                    for kk in range(k):
                        nc.scalar.dma_start(out=o_v[io, :, kk, :], in_=xt[:, kk, :])
                else:
                    nc.scalar.dma_start(out=o_v[io, :, :, :], in_=xt[:])

    nc.compile()
    return nc


def make_in_maps(images, e, t):
    x = np.asarray(images, dtype=np.float32).reshape(B, PIX).astype(np.float16)
    yy = np.asarray(e, dtype=np.float32).reshape(B, PIX).astype(np.float16)
    tt = np.ascontiguousarray(np.asarray(t, dtype=np.int32).reshape(B))
    in_maps = []
    for c in range(N_CORES):
        sl = slice(c * NS, (c + 1) * NS)
        in_maps.append(
            {
                "x": np.ascontiguousarray(x[sl]),
                "y": np.ascontiguousarray(yy[sl]),
                "t": np.ascontiguousarray(tt[sl]),
            }
        )
    return in_maps


def _get_runner():
    """Build (once) a jitted shard_map callable over the 8 cores.

    Mirrors concourse.bass2jax.run_bass_via_pjrt, but caches the compiled
    executable so repeated kernel() calls skip retracing/recompiling, and
    keeps the output placeholder buffers resident on device.
    """
    if "runner" in _cache:
        return _cache["runner"]

    import jax
    from jax.sharding import Mesh, PartitionSpec, NamedSharding
    from jax.experimental.shard_map import shard_map
    from concourse import mybir
    from concourse.bass2jax import (
        _bass_exec_p,
        install_neuronx_cc_hook,
        partition_id_tensor,
    )

    nc = _cache.get("nc")
    if nc is None:
        nc = _cache["nc"] = build_program()

    install_neuronx_cc_hook()

    partition_name = nc.partition_id_tensor.name if nc.partition_id_tensor else None
    in_names, out_names, out_avals = [], [], []
    for alloc in nc.m.functions[0].allocations:
        if not isinstance(alloc, mybir.MemoryLocationSet):
            continue
        name = alloc.memorylocations[0].name
        if alloc.kind == "ExternalInput":
            if name != partition_name:
                in_names.append(name)
        elif alloc.kind == "ExternalOutput":
            out_names.append(name)
            out_avals.append(
                jax.core.ShapedArray(tuple(alloc.tensor_shape), mybir.dt.np(alloc.dtype))
            )
    n_params = len(in_names)
    all_names = list(in_names) + out_names
    if partition_name is not None:
        all_names.append(partition_name)

    def _body(*args):
        # args = params + output placeholder buffers (the hook's parameter-
        # order check requires every bass_exec operand to be a jit parameter)
        operands = list(args)
        if partition_name is not None:
            operands.append(partition_id_tensor())
        outs = _bass_exec_p.bind(
            *operands,
            out_avals=tuple(out_avals),
            in_names=tuple(all_names),
            out_names=tuple(out_names),
            lowering_input_output_aliases=(),
            sim_require_finite=True,
            sim_require_nnan=True,
            nc=nc,
        )
        return tuple(outs)

    devices = jax.devices()[:N_CORES]
    assert len(devices) == N_CORES
    mesh = Mesh(np.asarray(devices), ("core",))
    n_outs = len(out_names)
    sharded = jax.jit(
        shard_map(
            _body,
            mesh=mesh,
            in_specs=(PartitionSpec("core"),) * (n_params + n_outs),
            out_specs=(PartitionSpec("core"),) * n_outs,
            check_rep=False,
        ),
        keep_unused=True,
    )
    # Output placeholder buffers: uploaded to device once, NOT donated, so
    # they stay valid and cost nothing on subsequent calls.
    zeros_dev = [
        jax.device_put(
            np.zeros((N_CORES * a.shape[0], *a.shape[1:]), a.dtype),
            NamedSharding(mesh, PartitionSpec("core")),
        )
        for a in out_avals
    ]
    _cache["runner"] = (sharded, in_names, out_names, zeros_dev)
    return _cache["runner"]


def kernel(images, e, t):
    images = np.asarray(images)
    orig_shape = images.shape

    x = np.asarray(images, dtype=np.float32).reshape(B, PIX).astype(np.float16)
    yy = np.asarray(e, dtype=np.float32).reshape(B, PIX).astype(np.float16)
    tt = np.ascontiguousarray(np.asarray(t, dtype=np.int32).reshape(B))

    try:
        sharded, in_names, out_names, zeros_dev = _get_runner()
        global_in = {"x": x, "y": yy, "t": tt}
        out_arrs = sharded(*[global_in[n] for n in in_names], *zeros_dev)
        full = np.asarray(out_arrs[out_names.index("out")])
    except Exception:
        # Fallback: the stock (slower, but battle-tested) execution path.
        from concourse import bass_utils

        if "nc" not in _cache:
            _cache["nc"] = build_program()
        res = bass_utils.run_bass_kernel_spmd(
            _cache["nc"], make_in_maps(images, e, t), core_ids=list(range(N_CORES))
        )
        full = np.concatenate([res.results[c]["out"] for c in range(N_CORES)], axis=0)

    return full.astype(np.float32).reshape(orig_shape)


# revision 4
# speedup vs baseline: 2.4732x; 1.1416x over previous
"""Trainium2 Bass kernel for the ConditionalDDPM forward-diffusion problem.

Computes  xt = sqrt(alpha_bar[t]) * images + sqrt(1 - alpha_bar[t]) * e
for B=65536 images of shape (1, 28, 28), t in [0, 1000).

Strategy (pure data parallel, 8 NeuronCores):
  - Shard images/e/t along batch: 8192 samples per core.
  - Pure HBM-bandwidth-bound (~358-435 GB/s/core).  The rel-err budget
    (2e-2) is exploited with reduced precision, quantized on the HOST with
    ml_dtypes (device only upconverts, exactly reproducible):
      images, e  ->  fp8 e3m4  (1 byte; measured global rel err 1.34e-2)
      out        ->  fp16      (2 bytes)
    Per-core traffic: 6.42 + 6.42 + 12.85 = 25.7MB vs 77MB in f32.
  - Per-sample scalars computed on device from t (no table gather):
    g(t) = ln(alpha_bar[t]) fitted by a degree-6 zero-intercept polynomial
    in u=(t+1)/1000 (f64 fit residual ~5e-13); a = exp(g/2), b = sqrt(1-e^g)
    in f32.  t is DMAed first on the sync queue so a/b are ready (~5us)
    before the first data tile lands.
  - Sample layout: sample s = 64*p + i lives at (partition p, unit i).
    Static SBUF tiles for x and e (no buffer recycling -> loads never wait).
  - Per unit i: scale u = a_i * x_i (ACT engine mostly, DVE for some), then
    combine out = (b_i * e_i) + u in-place into the u tile (DVE mostly,
    GPSIMD for ~24 mid-stream units - 3-way engine split so ~128 x 1us of
    elementwise work fits inside the ~61us DMA stream).
  - Stores: 4-unit groups on GPSIMD (SWDGE) for units 0-47; per-unit tail
    stores on SYNC (HWDGE) for units 48-63 to compress the end-of-stream
    dependency chain.
"""

import sys

if "/opt/trn_rl_repo" not in sys.path:
    sys.path.insert(0, "/opt/trn_rl_repo")

import numpy as np

B = 65536
T = 1000
BETA_1 = 1e-4
BETA_T = 0.02
N_CORES = 8
NS = B // N_CORES  # samples per core = 8192
PIX = 784
N_UNITS = NS // 128  # 64
LK = 8  # units per bulk load DMA
SK = 4  # units per bulk store DMA / u-tile group
BULK = 48  # units 0..47 bulk, 48..63 per-unit tail
POLY_DEG = 6

# engine split for the per-unit elementwise work: ACT does all 64 scale
# ops, DVE all 64 combines (Pool rejects TensorScalarPtr, so no gpsimd
# compute; gpsimd only triggers the bulk stores)
GP_COMBINE = frozenset()
DVE_SCALE = frozenset()

_cache = {}


def g_poly_coeffs() -> np.ndarray:
    """c[0..5] with g(u) ~= (((((c6*u + c5)*u + c4)*u + c3)*u + c2)*u + c1)*u,
    u = (t+1)/1000, g = ln(alpha_bar[t]).  Fit in f64; residual ~5e-13."""
    slope = (BETA_T - BETA_1) / (T - 1)
    betas = BETA_1 + slope * np.arange(T, dtype=np.float64)
    g_exact = np.cumsum(np.log1p(-betas))
    u = (np.arange(T, dtype=np.float64) + 1.0) / 1000.0
    A = np.stack([u**k for k in range(1, POLY_DEG + 1)], axis=1)
    c, *_ = np.linalg.lstsq(A, g_exact, rcond=None)
    return c


def _f8(arr32: np.ndarray) -> np.ndarray:
    import ml_dtypes

    return arr32.astype(ml_dtypes.float8_e3m4)


def build_program(ns: int = NS):
    """Build the per-core Bass program (same NEFF on all 8 cores)."""
    from concourse import bacc, mybir
    import concourse.tile as tile

    n_units = ns // 128
    f32 = mybir.dt.float32
    f16 = mybir.dt.float16
    f8 = mybir.dt.float8e3
    Alu = mybir.AluOpType
    Act = mybir.ActivationFunctionType
    coeffs = [float(c) for c in g_poly_coeffs()]

    nc = bacc.Bacc(
        "TRN2",
        target_bir_lowering=False,
        debug=False,
        enable_asserts=False,
        num_devices=N_CORES,
    )
    x = nc.dram_tensor("x", [ns, PIX], f8, kind="ExternalInput").ap()
    y = nc.dram_tensor("y", [ns, PIX], f8, kind="ExternalInput").ap()
    tt = nc.dram_tensor("t", [ns], mybir.dt.int32, kind="ExternalInput").ap()
    out = nc.dram_tensor("out", [ns, PIX], f16, kind="ExternalOutput").ap()

    # sample s = 64*p + i  ->  (partition p, unit i)
    x_v = x.rearrange("(p i) m -> p i m", p=128)
    y_v = y.rearrange("(p i) m -> p i m", p=128)
    o_v = out.rearrange("(p i) m -> p i m", p=128)
    t_v = tt.rearrange("(p i) -> p i", p=128)  # contiguous 256B per partition

    def scale_unit(i, out_ap, in_ap, a_t):
        if i in DVE_SCALE:
            nc.vector.tensor_scalar_mul(
                out=out_ap, in0=in_ap, scalar1=a_t[:, i : i + 1]
            )
        else:
            nc.scalar.activation(
                out=out_ap, in_=in_ap, func=Act.Copy, scale=a_t[:, i : i + 1]
            )

    def combine_unit(i, u_ap, e_ap, b_t):
        eng = nc.gpsimd if i in GP_COMBINE else nc.vector
        eng.scalar_tensor_tensor(
            out=u_ap,
            in0=e_ap,
            scalar=b_t[:, i : i + 1],
            in1=u_ap,
            op0=Alu.mult,
            op1=Alu.add,
        )

    with tile.TileContext(nc) as tc:
        with (
            tc.tile_pool(name="xs", bufs=1) as xpool,
            tc.tile_pool(name="ys", bufs=1) as ypool,
            tc.tile_pool(name="us", bufs=6) as upool,
            tc.tile_pool(name="ut", bufs=8) as utail,
            tc.tile_pool(name="singles", bufs=1) as singles,
        ):
            # ---- t load first (sync queue) so scalars are ready early ----
            ti = singles.tile([128, n_units], mybir.dt.int32)
            nc.sync.dma_start(out=ti[:], in_=t_v)

            # ---- per-sample scalars: a = exp(g/2), b = sqrt(1 - exp(g)) ----
            # u = (t + 1) / 1000   (int32 in, f32 out)
            uu = singles.tile([128, n_units], f32)
            nc.vector.tensor_scalar(
                out=uu[:], in0=ti[:], scalar1=1.0, scalar2=0.001,
                op0=Alu.add, op1=Alu.mult,
            )
            # Horner with zero intercept: h = u*c6; h = (h + c_k)*u, k=5..1
            hh = singles.tile([128, n_units], f32)
            nc.vector.tensor_scalar_mul(out=hh[:], in0=uu[:], scalar1=coeffs[5])
            for kk_ in range(POLY_DEG - 2, -1, -1):
                nc.vector.scalar_tensor_tensor(
                    out=hh[:], in0=hh[:], scalar=coeffs[kk_], in1=uu[:],
                    op0=Alu.add, op1=Alu.mult,
                )
            # a = exp(0.5*g)
            a_t = singles.tile([128, n_units], f32)
            nc.scalar.activation(out=a_t[:], in_=hh[:], func=Act.Exp, scale=0.5)
            # b = sqrt(1 - exp(g))
            b_t = singles.tile([128, n_units], f32)
            nc.scalar.activation(out=b_t[:], in_=hh[:], func=Act.Exp)
            nc.vector.tensor_scalar(
                out=b_t[:], in0=b_t[:], scalar1=1.0, scalar2=-1.0,
                op0=Alu.subtract, op1=Alu.mult,
            )
            nc.scalar.activation(out=b_t[:], in_=b_t[:], func=Act.Sqrt)

            # ---- static input tiles: loads never wait on anything ----
            x_sb = xpool.tile([128, n_units, PIX], f8)
            e_sb = ypool.tile([128, n_units, PIX], f8)
            # bulk loads, LK units per DMA (x/e interleaved in unit order)
            for c in range(BULK // LK):
                i0 = c * LK
                nc.sync.dma_start(out=x_sb[:, i0 : i0 + LK, :], in_=x_v[:, i0 : i0 + LK, :])
                nc.sync.dma_start(out=e_sb[:, i0 : i0 + LK, :], in_=y_v[:, i0 : i0 + LK, :])
            # per-unit tail loads
            for i in range(BULK, n_units):
                nc.sync.dma_start(out=x_sb[:, i, :], in_=x_v[:, i, :])
                nc.sync.dma_start(out=e_sb[:, i, :], in_=y_v[:, i, :])

            # ---- bulk compute + stores: SK-unit u tiles, stores on gpsimd ----
            for g in range(BULK // SK):
                i0 = g * SK
                u4 = upool.tile([128, SK, PIX], f16)
                for kk in range(SK):
                    i = i0 + kk
                    scale_unit(i, u4[:, kk, :], x_sb[:, i, :], a_t)
                    combine_unit(i, u4[:, kk, :], e_sb[:, i, :], b_t)
                nc.gpsimd.dma_start(out=o_v[:, i0 : i0 + SK, :], in_=u4[:])

            # ---- tail: per-unit compute + per-unit stores on sync ----
            for i in range(BULK, n_units):
                u1 = utail.tile([128, PIX], f16)
                scale_unit(i, u1[:], x_sb[:, i, :], a_t)
                combine_unit(i, u1[:], e_sb[:, i, :], b_t)
                nc.sync.dma_start(out=o_v[:, i, :], in_=u1[:])

    nc.compile()
    return nc


def make_in_maps(images, e, t):
    x = _f8(np.asarray(images, dtype=np.float32).reshape(B, PIX))
    yy = _f8(np.asarray(e, dtype=np.float32).reshape(B, PIX))
    tt = np.ascontiguousarray(np.asarray(t, dtype=np.int32).reshape(B))
    in_maps = []
    for c in range(N_CORES):
        sl = slice(c * NS, (c + 1) * NS)
        in_maps.append(
            {
                "x": np.ascontiguousarray(x[sl]),
                "y": np.ascontiguousarray(yy[sl]),
                "t": np.ascontiguousarray(tt[sl]),
            }
        )
    return in_maps


def _get_runner():
    """Build (once) a jitted shard_map callable over the 8 cores.

    Mirrors concourse.bass2jax.run_bass_via_pjrt, but caches the compiled
    executable so repeated kernel() calls skip retracing/recompiling, and
    keeps the output placeholder buffers resident on device.
    """
    if "runner" in _cache:
        return _cache["runner"]

    import jax
    from jax.sharding import Mesh, PartitionSpec, NamedSharding
    from jax.experimental.shard_map import shard_map
    from concourse import mybir
    from concourse.bass2jax import (
        _bass_exec_p,
        install_neuronx_cc_hook,
        partition_id_tensor,
    )

    nc = _cache.get("nc")
    if nc is None:
        nc = _cache["nc"] = build_program()

    install_neuronx_cc_hook()

    partition_name = nc.partition_id_tensor.name if nc.partition_id_tensor else None
    in_names, out_names, out_avals = [], [], []
    for alloc in nc.m.functions[0].allocations:
        if not isinstance(alloc, mybir.MemoryLocationSet):
            continue
        name = alloc.memorylocations[0].name
        if alloc.kind == "ExternalInput":
            if name != partition_name:
                in_names.append(name)
        elif alloc.kind == "ExternalOutput":
            out_names.append(name)
            out_avals.append(
                jax.core.ShapedArray(tuple(alloc.tensor_shape), mybir.dt.np(alloc.dtype))
            )
    n_params = len(in_names)
    all_names = list(in_names) + out_names
    if partition_name is not None:
        all_names.append(partition_name)

    def _body(*args):
        # args = params + output placeholder buffers (the hook's parameter-
        # order check requires every bass_exec operand to be a jit parameter)
        operands = list(args)
        if partition_name is not None:
            operands.append(partition_id_tensor())
        outs = _bass_exec_p.bind(
            *operands,
            out_avals=tuple(out_avals),
            in_names=tuple(all_names),
            out_names=tuple(out_names),
            lowering_input_output_aliases=(),
            sim_require_finite=True,
            sim_require_nnan=True,
            nc=nc,
        )
        return tuple(outs)

    devices = jax.devices()[:N_CORES]
    assert len(devices) == N_CORES
    mesh = Mesh(np.asarray(devices), ("core",))
    n_outs = len(out_names)
    sharded = jax.jit(
        shard_map(
            _body,
            mesh=mesh,
            in_specs=(PartitionSpec("core"),) * (n_params + n_outs),
            out_specs=(PartitionSpec("core"),) * n_outs,
            check_rep=False,
        ),
        keep_unused=True,
    )
    # Output placeholder buffers: uploaded to device once, NOT donated, so
    # they stay valid and cost nothing on subsequent calls.
    zeros_dev = [
        jax.device_put(
            np.zeros((N_CORES * a.shape[0], *a.shape[1:]), a.dtype),
            NamedSharding(mesh, PartitionSpec("core")),
        )
        for a in out_avals
    ]
    _cache["runner"] = (sharded, in_names, out_names, zeros_dev)
    return _cache["runner"]


def kernel(images, e, t):
    images = np.asarray(images)
    orig_shape = images.shape

    x = _f8(images.astype(np.float32, copy=False).reshape(B, PIX))
    yy = _f8(np.asarray(e, dtype=np.float32).reshape(B, PIX))
    tt = np.ascontiguousarray(np.asarray(t, dtype=np.int32).reshape(B))

    try:
        sharded, in_names, out_names, zeros_dev = _get_runner()
        global_in = {"x": x, "y": yy, "t": tt}
        out_arrs = sharded(*[global_in[n] for n in in_names], *zeros_dev)
        full = np.asarray(out_arrs[out_names.index("out")])
    except Exception:
        # Fallback: the stock (slower, but battle-tested) execution path.
        from concourse import bass_utils

        if "nc" not in _cache:
            _cache["nc"] = build_program()
        res = bass_utils.run_bass_kernel_spmd(
            _cache["nc"], make_in_maps(images, e, t), core_ids=list(range(N_CORES))
        )
        full = np.concatenate([res.results[c]["out"] for c in range(N_CORES)], axis=0)

    return full.astype(np.float32).reshape(orig_shape)
